# revision 6
# baseline (speedup 1.0000x reference)
"""Trainium2 Bass kernel for nn_GAT_87617332838818.

Mathematical collapse: the reference GAT aggregates ``alpha * hp[:, dst]``
over incoming edges per destination node.  Since the softmax weights alpha
sum to exactly 1 within each destination segment and the aggregated message
``hp[dst]`` is constant within the segment, the whole message-passing step
is the identity: ``out[n] = hp[n]``.  The network therefore reduces to a
per-node 3-layer MLP:

    logits = W2r @ elu(W1r @ elu(W0r @ x^T))        (per node column)

with W0r = W0.reshape(96,128), W1r = W1.reshape(96,96), W2r = W2.reshape(40,96)
(head-concat order matches the plain reshape).  Verified numerically against
the reference: rel fro err 4e-7 in f32; 4.5e-3 with this device pipeline.

Device strategy (8 NeuronCores, node-sharded 6250 rows each):
  - activations kept feature-on-partition: xT [128, n], h [96, n]
  - ELU via the split  elu(p') + 1 = max(p',0) + min(exp(p'),1)  with
    p' = p + nb (nb folds the "+1" inflation of the previous layer:
    nb = -W @ ones).  r = max(p+nb,0) and t = min(exp(p+nb),1) are fed
    through TWO accumulating matmuls (linearity), so the inflated h+1 is
    only ever formed in f32 PSUM — bf16-safe.
  - final layer bias cb2 = W2 @ ones subtracted in the output drain pass.
  - per 512-column group: PE 5 matmuls, ACT exp x2, PSUM drains (r x2 +
    out x1) split between DVE and ACT for balance, t=min(e,1) on GpSimd.
  - 3-stage software-pipelined emission so each engine's in-order stream
    always has ready work (avoids head-of-line blocking across groups).
"""

import os
import sys

import numpy as np

for _p in ("/root/.axon_site/_ro/trn_rl_repo", "/opt/trn_rl_repo"):
    if os.path.isdir(_p) and _p not in sys.path:
        sys.path.append(_p)

import concourse.bass as bass
import concourse.tile as tile
from concourse import bacc, mybir
from concourse.bass_utils import run_bass_kernel_spmd

N_CORES = 8
N_PER = 6250            # 50000 / 8
D_IN = 128
D_HID = 96
D_OUT = 40
FD = 512                # group free-dim (1 PSUM bank)

F16 = mybir.dt.float16
BF16 = mybir.dt.bfloat16
F32 = mybir.dt.float32

Act = mybir.ActivationFunctionType
Alu = mybir.AluOpType

_groups = [FD] * (N_PER // FD)
if N_PER % FD:
    _groups.append(N_PER % FD)
G = len(_groups)
_starts = [sum(_groups[:i]) for i in range(G)]

# drain assignment: which r/out PSUM drains go to ACT instead of DVE.
# r-drains: 26 total (2 per group), out-drains: 13. ACT also does 26 exps.
R_DRAIN_ON_ACT = ()          # set of (group, layer) pairs
OUT_DRAIN_ON_ACT = ()        # set of groups
T_ON_GPSIMD = True


def _build_program() -> bass.Bass:
    nc = bacc.Bacc(None, target_bir_lowering=False, debug=False)

    xT = nc.declare_dram_parameter("xT", [D_IN, N_PER], F16, isOutput=False)
    w0t = nc.declare_dram_parameter("w0t", [D_IN, D_HID], F16, isOutput=False)
    w1t = nc.declare_dram_parameter("w1t", [D_HID, D_HID], BF16, isOutput=False)
    w2t = nc.declare_dram_parameter("w2t", [D_HID, D_OUT], BF16, isOutput=False)
    nb1 = nc.declare_dram_parameter("nb1", [D_HID, 1], F32, isOutput=False)
    ncb2 = nc.declare_dram_parameter("ncb2", [D_OUT, 1], F32, isOutput=False)
    yT = nc.declare_dram_parameter("yT", [D_OUT, N_PER], F16, isOutput=True)

    st = {}  # per-group pipeline state

    with tile.TileContext(nc) as tc:
        with (
            tc.tile_pool(name="consts", bufs=1) as consts,
            tc.tile_pool(name="xin", bufs=4) as xpool,
            tc.tile_pool(name="sb", bufs=3) as sb,
            tc.tile_pool(name="ps0", bufs=3, space="PSUM") as ps0,
            tc.tile_pool(name="ps1", bufs=3, space="PSUM") as ps1,
            tc.tile_pool(name="ps2", bufs=2, space="PSUM") as ps2,
        ):
            w0_sb = consts.tile([D_IN, D_HID], F16, tag="w0")
            w1_sb = consts.tile([D_HID, D_HID], BF16, tag="w1")
            w2_sb = consts.tile([D_HID, D_OUT], BF16, tag="w2")
            nb1_sb = consts.tile([D_HID, 1], F32, tag="nb1")
            ncb2_sb = consts.tile([D_OUT, 1], F32, tag="ncb2")
            nc.sync.dma_start(w0_sb[:], w0t[:])
            nc.sync.dma_start(w1_sb[:], w1t[:])
            nc.sync.dma_start(w2_sb[:], w2t[:])
            nc.sync.dma_start(nb1_sb[:], nb1[:])
            nc.sync.dma_start(ncb2_sb[:], ncb2[:])

            def drain(out_ap, psum_ap, bias_ap, on_act):
                """out = psum + bias (bias may be None), PSUM -> SBUF."""
                if on_act:
                    if bias_ap is None:
                        nc.scalar.activation(out_ap, psum_ap, Act.Copy)
                    else:
                        nc.scalar.activation(out_ap, psum_ap, Act.Identity,
                                             bias=bias_ap)
                else:
                    if bias_ap is None:
                        nc.vector.tensor_copy(out_ap, psum_ap)
                    else:
                        nc.vector.tensor_scalar_add(out_ap, psum_ap, bias_ap)

            def relu_drain(out_ap, psum_ap, bias_ap, on_act):
                """out = max(psum + bias, 0), PSUM -> SBUF bf16."""
                if on_act:
                    nc.scalar.activation(out_ap, psum_ap, Act.Relu,
                                         bias=(bias_ap if bias_ap is not None
                                               else 0.0))
                elif bias_ap is None:
                    nc.vector.tensor_scalar_max(out_ap, psum_ap, 0.0)
                else:
                    nc.vector.tensor_scalar(out_ap, psum_ap, bias_ap, 0.0,
                                            Alu.add, Alu.max)

            def exp_elu(g, lyr, p, fd, bias_ap):
                """From psum p: e=exp(p+nb), r=max(p+nb,0), t=min(e,1)."""
                e = sb.tile([D_HID, FD], BF16, tag=f"e{lyr}")
                if bias_ap is None:
                    nc.scalar.activation(e[:, :fd], p[:, :fd], Act.Exp)
                else:
                    nc.scalar.activation(e[:, :fd], p[:, :fd], Act.Exp,
                                         bias=bias_ap)
                r = sb.tile([D_HID, FD], BF16, tag=f"r{lyr}")
                relu_drain(r[:, :fd], p[:, :fd], bias_ap,
                           (g, lyr) in R_DRAIN_ON_ACT)
                t = sb.tile([D_HID, FD], BF16, tag=f"t{lyr}")
                eng = nc.gpsimd if T_ON_GPSIMD else nc.vector
                eng.tensor_scalar_min(t[:, :fd], e[:, :fd], 1.0)
                return r, t

            def stage_load(g):
                fd = _groups[g]
                xt = xpool.tile([D_IN, FD], F16, tag="xt")
                nc.sync.dma_start(xt[:, :fd], xT[:, _starts[g]:_starts[g] + fd])
                st[g] = {"xt": xt}

            def stage0(g):
                fd = _groups[g]
                s = st[g]
                p0 = ps0.tile([D_HID, FD], F32, tag="p0")
                nc.tensor.matmul(p0[:, :fd], w0_sb[:], s["xt"][:, :fd],
                                 start=True, stop=True)
                s["r1"], s["t1"] = exp_elu(g, 0, p0, fd, None)

            def stage1(g):
                fd = _groups[g]
                s = st[g]
                p1 = ps1.tile([D_HID, FD], F32, tag="p1")
                nc.tensor.matmul(p1[:, :fd], w1_sb[:], s["r1"][:, :fd],
                                 start=True, stop=False)
                nc.tensor.matmul(p1[:, :fd], w1_sb[:], s["t1"][:, :fd],
                                 start=False, stop=True)
                s["r2"], s["t2"] = exp_elu(g, 1, p1, fd, nb1_sb[:])

            def stage2(g):
                fd = _groups[g]
                s = st.pop(g)
                p2 = ps2.tile([D_OUT, FD], F32, tag="p2")
                nc.tensor.matmul(p2[:, :fd], w2_sb[:], s["r2"][:, :fd],
                                 start=True, stop=False)
                nc.tensor.matmul(p2[:, :fd], w2_sb[:], s["t2"][:, :fd],
                                 start=False, stop=True)
                o = sb.tile([D_OUT, FD], F16, tag="o")
                drain(o[:, :fd], p2[:, :fd], ncb2_sb[:],
                      g in OUT_DRAIN_ON_ACT)
                nc.sync.dma_start(yT[:, _starts[g]:_starts[g] + fd],
                                  o[:, :fd])

            # software-pipelined emission: keeps each engine's in-order
            # stream supplied with ready work across groups.
            for gg in range(G + 3):
                if gg < G:
                    stage_load(gg)
                if 0 <= gg - 1 < G:
                    stage0(gg - 1)
                if 0 <= gg - 2 < G:
                    stage1(gg - 2)
                if 0 <= gg - 3 < G:
                    stage2(gg - 3)

    nc.compile()
    return nc


_prog_cache = []
last_result = None


def kernel(**inputs) -> np.ndarray:
    global last_result
    x = np.asarray(inputs["x"], np.float32)           # [50000, 128]
    W0 = np.asarray(inputs["W0"], np.float32).reshape(D_HID, D_IN)
    W1 = np.asarray(inputs["W1"], np.float32).reshape(D_HID, D_HID)
    W2 = np.asarray(inputs["W2"], np.float32).reshape(D_OUT, D_HID)

    n = x.shape[0]
    assert n == N_CORES * N_PER, f"unexpected node count {n}"

    import ml_dtypes
    xT16 = np.ascontiguousarray(x.T.astype(np.float16))      # [128, 50000]
    w0t = np.ascontiguousarray(W0.T.astype(np.float16))      # [128, 96]
    w1tb = np.ascontiguousarray(W1.T.astype(ml_dtypes.bfloat16))
    w2tb = np.ascontiguousarray(W2.T.astype(ml_dtypes.bfloat16))
    w1f = w1tb.astype(np.float32)
    w2f = w2tb.astype(np.float32)
    nb1 = -w1f.sum(axis=0, keepdims=True).T.astype(np.float32)   # -(W1 @ 1)
    ncb2 = -w2f.sum(axis=0, keepdims=True).T.astype(np.float32)  # -(W2 @ 1)

    if not _prog_cache:
        _prog_cache.append(_build_program())
    nc = _prog_cache[0]

    in_maps = [
        dict(
            xT=np.ascontiguousarray(xT16[:, i * N_PER:(i + 1) * N_PER]),
            w0t=w0t, w1t=w1tb, w2t=w2tb, nb1=nb1, ncb2=ncb2,
        )
        for i in range(N_CORES)
    ]
    res = run_bass_kernel_spmd(nc, in_maps, list(range(N_CORES)))
    last_result = res
    out = np.concatenate(
        [np.asarray(res.results[i]["yT"], np.float32).T for i in range(N_CORES)],
        axis=0,
    )
    return out


if __name__ == "__main__":
    data = np.load("/tmp/gat_inputs.npz")
    y = kernel(**{k: data[k] for k in data.files})
    print("out", y.shape, y.dtype, "absmax", np.abs(y).max())


# revision 10
# speedup vs baseline: 3.8163x; 3.8163x over previous
"""Trainium2 Bass kernel for nn_GAT_87617332838818.

Mathematical collapse: the reference GAT aggregates ``alpha * hp[:, dst]``
over incoming edges per destination node.  Since the softmax weights alpha
sum to exactly 1 within each destination segment and the aggregated message
``hp[dst]`` is constant within the segment, the whole message-passing step
is the identity: ``out[n] = hp[n]``.  The network therefore reduces to a
per-node 3-layer MLP:

    logits = W2r @ elu(W1r @ elu(W0r @ x^T))        (per node column)

with W0r = W0.reshape(96,128), W1r = W1.reshape(96,96), W2r = W2.reshape(40,96)
(head-concat order matches the plain reshape).  Verified numerically against
the reference: rel fro err 4e-7 in f32; 4.5e-3 with this device pipeline.

Device strategy (8 NeuronCores, node-sharded 6250 rows each):
  - activations kept feature-on-partition: xT [128, n], h [96, n]
  - ELU via the split  elu(p') + 1 = max(p',0) + min(exp(p'),1)  with
    p' = p + nb (nb folds the "+1" inflation of the previous layer:
    nb = -W @ ones).  r = max(p+nb,0) and t = min(exp(p+nb),1) are fed
    through TWO accumulating matmuls (linearity), so the inflated h+1 is
    only ever formed in f32 PSUM — bf16-safe.
  - final layer bias cb2 = W2 @ ones subtracted in the output drain pass.
  - per 512-column group: PE 5 matmuls, ACT exp x2, PSUM drains (r x2 +
    out x1) split between DVE and ACT for balance, t=min(e,1) on GpSimd.
  - 3-stage software-pipelined emission so each engine's in-order stream
    always has ready work (avoids head-of-line blocking across groups).
"""

import os
import sys

import numpy as np

for _p in ("/root/.axon_site/_ro/trn_rl_repo", "/opt/trn_rl_repo"):
    if os.path.isdir(_p) and _p not in sys.path:
        sys.path.append(_p)

import concourse.bass as bass
import concourse.tile as tile
from concourse import bacc, mybir
from concourse.bass_utils import run_bass_kernel_spmd

N_CORES = 8
N_PER = 6250            # 50000 / 8
D_IN = 128
D_HID = 96
D_OUT = 40
FD = 512                # group free-dim (1 PSUM bank)

F16 = mybir.dt.float16
BF16 = mybir.dt.bfloat16
F32 = mybir.dt.float32

Act = mybir.ActivationFunctionType
Alu = mybir.AluOpType

_groups = [FD] * (N_PER // FD)
if N_PER % FD:
    _groups.append(N_PER % FD)
G = len(_groups)
_starts = [sum(_groups[:i]) for i in range(G)]

# drain assignment: which r/out PSUM drains go to ACT instead of DVE.
# r-drains: 26 total (2 per group), out-drains: 13. ACT also does 26 exps.
R_DRAIN_ON_ACT = tuple((g, 0) for g in range(G))   # L0 r-drains on ACT
OUT_DRAIN_ON_ACT = tuple(g for g in range(G) if g % 2 == 0)
T_ON_GPSIMD = False
DMA_BATCH = 3                # groups per input DMA


def _build_program() -> bass.Bass:
    nc = bacc.Bacc(None, target_bir_lowering=False, debug=False)

    xT = nc.declare_dram_parameter("xT", [D_IN, N_PER], F16, isOutput=False)
    w0t = nc.declare_dram_parameter("w0t", [D_IN, D_HID], F16, isOutput=False)
    w1t = nc.declare_dram_parameter("w1t", [D_HID, D_HID], BF16, isOutput=False)
    w2t = nc.declare_dram_parameter("w2t", [D_HID, D_OUT], BF16, isOutput=False)
    nb1 = nc.declare_dram_parameter("nb1", [D_HID, 1], F32, isOutput=False)
    ncb2 = nc.declare_dram_parameter("ncb2", [D_OUT, 1], F32, isOutput=False)
    yT = nc.declare_dram_parameter("yT", [D_OUT, N_PER], F16, isOutput=True)

    st = {}  # per-group pipeline state

    with tile.TileContext(nc) as tc:
        with (
            tc.tile_pool(name="consts", bufs=1) as consts,
            tc.tile_pool(name="xin", bufs=4) as xpool,
            tc.tile_pool(name="sb", bufs=3) as sb,
            tc.tile_pool(name="ps0", bufs=3, space="PSUM") as ps0,
            tc.tile_pool(name="ps1", bufs=3, space="PSUM") as ps1,
            tc.tile_pool(name="ps2", bufs=2, space="PSUM") as ps2,
        ):
            w0_sb = consts.tile([D_IN, D_HID], F16, tag="w0")
            w1_sb = consts.tile([D_HID, D_HID], BF16, tag="w1")
            w2_sb = consts.tile([D_HID, D_OUT], BF16, tag="w2")
            nb1_sb = consts.tile([D_HID, 1], F32, tag="nb1")
            ncb2_sb = consts.tile([D_OUT, 1], F32, tag="ncb2")
            nc.sync.dma_start(w0_sb[:], w0t[:])
            nc.sync.dma_start(w1_sb[:], w1t[:])
            nc.sync.dma_start(w2_sb[:], w2t[:])
            nc.sync.dma_start(nb1_sb[:], nb1[:])
            nc.sync.dma_start(ncb2_sb[:], ncb2[:])

            def drain(out_ap, psum_ap, bias_ap, on_act):
                """out = psum + bias (bias may be None), PSUM -> SBUF."""
                if on_act:
                    if bias_ap is None:
                        nc.scalar.activation(out_ap, psum_ap, Act.Copy)
                    else:
                        nc.scalar.activation(out_ap, psum_ap, Act.Identity,
                                             bias=bias_ap)
                else:
                    if bias_ap is None:
                        nc.vector.tensor_copy(out_ap, psum_ap)
                    else:
                        nc.vector.tensor_scalar_add(out_ap, psum_ap, bias_ap)

            def relu_drain(out_ap, psum_ap, bias_ap, on_act):
                """out = max(psum + bias, 0), PSUM -> SBUF bf16."""
                if on_act:
                    nc.scalar.activation(out_ap, psum_ap, Act.Relu,
                                         bias=(bias_ap if bias_ap is not None
                                               else 0.0))
                elif bias_ap is None:
                    nc.vector.tensor_scalar_max(out_ap, psum_ap, 0.0)
                else:
                    nc.vector.tensor_scalar(out_ap, psum_ap, bias_ap, 0.0,
                                            Alu.add, Alu.max)

            def exp_elu(g, lyr, p, fd, bias_ap):
                """From psum p: e=exp(p+nb), r=max(p+nb,0), t=min(e,1)."""
                e = sb.tile([D_HID, FD], BF16, tag=f"e{lyr}")
                if bias_ap is None:
                    nc.scalar.activation(e[:, :fd], p[:, :fd], Act.Exp)
                else:
                    nc.scalar.activation(e[:, :fd], p[:, :fd], Act.Exp,
                                         bias=bias_ap)
                r = sb.tile([D_HID, FD], BF16, tag=f"r{lyr}")
                relu_drain(r[:, :fd], p[:, :fd], bias_ap,
                           (g, lyr) in R_DRAIN_ON_ACT)
                t = sb.tile([D_HID, FD], BF16, tag=f"t{lyr}")
                eng = nc.gpsimd if T_ON_GPSIMD else nc.vector
                eng.tensor_scalar_min(t[:, :fd], e[:, :fd], 1.0)
                return r, t

            def stage_load(g):
                if g % DMA_BATCH:
                    return
                hi = min(g + DMA_BATCH, G)
                cols = _starts[hi - 1] + _groups[hi - 1] - _starts[g]
                xt = xpool.tile([D_IN, FD * DMA_BATCH], F16, tag="xt")
                nc.sync.dma_start(xt[:, :cols],
                                  xT[:, _starts[g]:_starts[g] + cols])
                for gi in range(g, hi):
                    st[gi] = {"xt": xt, "xoff": _starts[gi] - _starts[g]}

            def stage0(g):
                fd = _groups[g]
                s = st[g]
                p0 = ps0.tile([D_HID, FD], F32, tag="p0")
                xo = s["xoff"]
                nc.tensor.matmul(p0[:, :fd], w0_sb[:],
                                 s["xt"][:, xo:xo + fd],
                                 start=True, stop=True)
                s["r1"], s["t1"] = exp_elu(g, 0, p0, fd, None)

            def stage1(g):
                fd = _groups[g]
                s = st[g]
                p1 = ps1.tile([D_HID, FD], F32, tag="p1")
                nc.tensor.matmul(p1[:, :fd], w1_sb[:], s["r1"][:, :fd],
                                 start=True, stop=False)
                nc.tensor.matmul(p1[:, :fd], w1_sb[:], s["t1"][:, :fd],
                                 start=False, stop=True)
                s["r2"], s["t2"] = exp_elu(g, 1, p1, fd, nb1_sb[:])

            def stage2(g):
                fd = _groups[g]
                s = st.pop(g)
                p2 = ps2.tile([D_OUT, FD], F32, tag="p2")
                nc.tensor.matmul(p2[:, :fd], w2_sb[:], s["r2"][:, :fd],
                                 start=True, stop=False)
                nc.tensor.matmul(p2[:, :fd], w2_sb[:], s["t2"][:, :fd],
                                 start=False, stop=True)
                o = sb.tile([D_OUT, FD], F16, tag="o")
                drain(o[:, :fd], p2[:, :fd], ncb2_sb[:],
                      g in OUT_DRAIN_ON_ACT)
                nc.gpsimd.dma_start(yT[:, _starts[g]:_starts[g] + fd],
                                    o[:, :fd])

            # software-pipelined emission: keeps each engine's in-order
            # stream supplied with ready work across groups.
            for gg in range(G + 3):
                if gg < G:
                    stage_load(gg)
                if 0 <= gg - 1 < G:
                    stage0(gg - 1)
                if 0 <= gg - 2 < G:
                    stage1(gg - 2)
                if 0 <= gg - 3 < G:
                    stage2(gg - 3)

    nc.compile()
    return nc


_prog_cache = []
last_result = None


def kernel(**inputs) -> np.ndarray:
    global last_result
    x = np.asarray(inputs["x"], np.float32)           # [50000, 128]
    W0 = np.asarray(inputs["W0"], np.float32).reshape(D_HID, D_IN)
    W1 = np.asarray(inputs["W1"], np.float32).reshape(D_HID, D_HID)
    W2 = np.asarray(inputs["W2"], np.float32).reshape(D_OUT, D_HID)

    n = x.shape[0]
    assert n == N_CORES * N_PER, f"unexpected node count {n}"

    import ml_dtypes
    xT16 = np.ascontiguousarray(x.T.astype(np.float16))      # [128, 50000]
    w0t = np.ascontiguousarray(W0.T.astype(np.float16))      # [128, 96]
    w1tb = np.ascontiguousarray(W1.T.astype(ml_dtypes.bfloat16))
    w2tb = np.ascontiguousarray(W2.T.astype(ml_dtypes.bfloat16))
    w1f = w1tb.astype(np.float32)
    w2f = w2tb.astype(np.float32)
    nb1 = -w1f.sum(axis=0, keepdims=True).T.astype(np.float32)   # -(W1 @ 1)
    ncb2 = -w2f.sum(axis=0, keepdims=True).T.astype(np.float32)  # -(W2 @ 1)

    if not _prog_cache:
        _prog_cache.append(_build_program())
    nc = _prog_cache[0]

    in_maps = [
        dict(
            xT=np.ascontiguousarray(xT16[:, i * N_PER:(i + 1) * N_PER]),
            w0t=w0t, w1t=w1tb, w2t=w2tb, nb1=nb1, ncb2=ncb2,
        )
        for i in range(N_CORES)
    ]
    res = run_bass_kernel_spmd(nc, in_maps, list(range(N_CORES)))
    last_result = res
    out = np.concatenate(
        [np.asarray(res.results[i]["yT"], np.float32).T for i in range(N_CORES)],
        axis=0,
    )
    return out


if __name__ == "__main__":
    data = np.load("/tmp/gat_inputs.npz")
    y = kernel(**{k: data[k] for k in data.files})
    print("out", y.shape, y.dtype, "absmax", np.abs(y).max())


# revision 14
# speedup vs baseline: 3.8594x; 1.0113x over previous
"""Trainium2 Bass kernel for nn_GAT_87617332838818.

Mathematical collapse: the reference GAT aggregates ``alpha * hp[:, dst]``
over incoming edges per destination node.  Since the softmax weights alpha
sum to exactly 1 within each destination segment and the aggregated message
``hp[dst]`` is constant within the segment, the whole message-passing step
is the identity: ``out[n] = hp[n]``.  The network therefore reduces to a
per-node 3-layer MLP:

    logits = W2r @ elu(W1r @ elu(W0r @ x^T))        (per node column)

with W0r = W0.reshape(96,128), W1r = W1.reshape(96,96), W2r = W2.reshape(40,96)
(head-concat order matches the plain reshape).  Verified numerically against
the reference: rel fro err 4e-7 in f32; 4.5e-3 with this device pipeline.

Device strategy (8 NeuronCores, node-sharded 6250 rows each):
  - activations kept feature-on-partition: xT [128, n], h [96, n]
  - ELU via the split  elu(p') + 1 = max(p',0) + min(exp(p'),1)  with
    p' = p + nb (nb folds the "+1" inflation of the previous layer:
    nb = -W @ ones).  r = max(p+nb,0) and t = min(exp(p+nb),1) are fed
    through TWO accumulating matmuls (linearity), so the inflated h+1 is
    only ever formed in f32 PSUM — bf16-safe.
  - final layer bias cb2 = W2 @ ones subtracted in the output drain pass.
  - per 512-column group: PE 5 matmuls, ACT exp x2 + relu drain, DVE
    r/t/out passes; PSUM drains split between DVE and ACT for balance.
  - 3-stage software-pipelined emission so each engine's in-order stream
    always has ready work (avoids head-of-line blocking across groups).
  - ~18 dummy matmuls parked in the DMA-bound head flip the PE HAM clock
    gate to 2.4 GHz before the real matmuls start (measured 427->216 ns).
  - w0 rides in the first x DMA batch; w1/w2 and biases are packed into
    single DMAs to cut ~620 ns/issue sequencer serialization.
"""

import os
import sys

import numpy as np

for _p in ("/root/.axon_site/_ro/trn_rl_repo", "/opt/trn_rl_repo"):
    if os.path.isdir(_p) and _p not in sys.path:
        sys.path.append(_p)

import concourse.bass as bass
import concourse.tile as tile
from concourse import bacc, mybir
from concourse.bass_utils import run_bass_kernel_spmd

N_CORES = 8
N_PER = 6250            # 50000 / 8
D_IN = 128
D_HID = 96
D_OUT = 40
FD = 512                # group free-dim (1 PSUM bank)

F16 = mybir.dt.float16
BF16 = mybir.dt.bfloat16
F32 = mybir.dt.float32

Act = mybir.ActivationFunctionType
Alu = mybir.AluOpType

_groups = [FD] * (N_PER // FD)
if N_PER % FD:
    _groups.append(N_PER % FD)
G = len(_groups)
_starts = [sum(_groups[:i]) for i in range(G)]

# drain assignment: which r/out PSUM drains go to ACT instead of DVE.
R_DRAIN_ON_ACT = tuple((g, 0) for g in range(G))   # L0 r-drains on ACT
OUT_DRAIN_ON_ACT = tuple(g for g in range(G) if g % 2 == 0)
X_BATCHES = [1, 4, 4, 4]     # groups per input DMA (first small -> fast start)
N_WARMUP_MM = 18             # dummy matmuls to flip the PE HAM to 2.4 GHz

_batch_of = {}
_b0 = 0
for _bi, _bn in enumerate(X_BATCHES):
    for _g in range(_b0, min(_b0 + _bn, G)):
        _batch_of[_g] = _bi
    _b0 += _bn
assert _b0 >= G


def _build_program() -> bass.Bass:
    nc = bacc.Bacc(None, target_bir_lowering=False, debug=False)

    # xw packs [w0t | xT]: cols 0..95 = W0^T fp16, cols 96.. = x^T shard
    xw = nc.declare_dram_parameter("xw", [D_IN, D_HID + N_PER], F16,
                                   isOutput=False)
    # wb packs [w1t | w2t] bf16
    wb = nc.declare_dram_parameter("wb", [D_HID, D_HID + D_OUT], BF16,
                                   isOutput=False)
    # biases: col 0 = nb1 (= -W1@1), col 1 rows 0..39 = -cb2 (= -W2@1)
    bias = nc.declare_dram_parameter("bias", [D_HID, 2], F32, isOutput=False)
    yT = nc.declare_dram_parameter("yT", [D_OUT, N_PER], F16, isOutput=True)

    st = {}  # per-group pipeline state
    batch_tiles = {}

    with tile.TileContext(nc) as tc:
        with (
            tc.tile_pool(name="consts", bufs=1) as consts,
            tc.tile_pool(name="x0", bufs=1) as x0pool,
            tc.tile_pool(name="xin", bufs=2) as xpool,
            tc.tile_pool(name="sb", bufs=3) as sb,
            tc.tile_pool(name="ps0", bufs=3, space="PSUM") as ps0,
            tc.tile_pool(name="ps1", bufs=3, space="PSUM") as ps1,
            tc.tile_pool(name="ps2", bufs=2, space="PSUM") as ps2,
        ):
            # --- PE warm-up: dummy matmuls on garbage SBUF during the
            # DMA-bound head; no data deps, output overwritten later.
            junk_w = consts.tile([D_IN, D_OUT], F16, tag="junkw")
            junk_x = consts.tile([D_IN, FD], F16, tag="junkx")
            nc.gpsimd.memset(junk_w[:], 0.0)
            nc.gpsimd.memset(junk_x[:], 0.0)
            warm = ps2.tile([D_OUT, FD], F32, tag="p2")
            for _ in range(N_WARMUP_MM):
                nc.tensor.matmul(warm[:], junk_w[:], junk_x[:],
                                 start=True, stop=True)

            wb_sb = consts.tile([D_HID, D_HID + D_OUT], BF16, tag="wb")
            bias_sb = consts.tile([D_HID, 2], F32, tag="bias")
            nc.sync.dma_start(wb_sb[:], wb[:])
            nc.sync.dma_start(bias_sb[:], bias[:])
            w1_sb = wb_sb[:, :D_HID]
            w2_sb = wb_sb[:, D_HID:D_HID + D_OUT]
            nb1_sb = bias_sb[:, 0:1]
            ncb2_sb = bias_sb[:D_OUT, 1:2]

            def drain(out_ap, psum_ap, bias_ap, on_act):
                """out = psum + bias, PSUM -> SBUF."""
                if on_act:
                    nc.scalar.activation(out_ap, psum_ap, Act.Identity,
                                         bias=bias_ap)
                else:
                    nc.vector.tensor_scalar_add(out_ap, psum_ap, bias_ap)

            def relu_drain(out_ap, psum_ap, bias_ap, on_act):
                """out = max(psum + bias, 0), PSUM -> SBUF bf16."""
                if on_act:
                    nc.scalar.activation(out_ap, psum_ap, Act.Relu,
                                         bias=(bias_ap if bias_ap is not None
                                               else 0.0))
                elif bias_ap is None:
                    nc.vector.tensor_scalar_max(out_ap, psum_ap, 0.0)
                else:
                    nc.vector.tensor_scalar(out_ap, psum_ap, bias_ap, 0.0,
                                            Alu.add, Alu.max)

            def exp_elu(g, lyr, p, fd, bias_ap):
                """From psum p: e=exp(p+nb), r=max(p+nb,0), t=min(e,1)."""
                e = sb.tile([D_HID, FD], BF16, tag=f"e{lyr}")
                if bias_ap is None:
                    nc.scalar.activation(e[:, :fd], p[:, :fd], Act.Exp)
                else:
                    nc.scalar.activation(e[:, :fd], p[:, :fd], Act.Exp,
                                         bias=bias_ap)
                r = sb.tile([D_HID, FD], BF16, tag=f"r{lyr}")
                relu_drain(r[:, :fd], p[:, :fd], bias_ap,
                           (g, lyr) in R_DRAIN_ON_ACT)
                t = sb.tile([D_HID, FD], BF16, tag=f"t{lyr}")
                nc.vector.tensor_scalar_min(t[:, :fd], e[:, :fd], 1.0)
                return r, t

            def stage_load(g):
                bi = _batch_of[g]
                if g > 0 and _batch_of[g - 1] == bi:
                    st[g] = st_batch[bi]
                    return
                g0 = g
                g1 = g0
                while g1 + 1 < G and _batch_of[g1 + 1] == bi:
                    g1 += 1
                lo = _starts[g0] + (0 if bi else -D_HID)  # batch 0 incl. w0
                hi = _starts[g1] + _groups[g1]
                cols = hi - lo
                pool = x0pool if bi == 0 else xpool
                width = D_HID + FD * X_BATCHES[0] if bi == 0 else FD * 4
                xt = pool.tile([D_IN, width], F16,
                               tag=("xt0" if bi == 0 else "xt"))
                nc.sync.dma_start(xt[:, :cols],
                                  xw[:, D_HID + lo:D_HID + hi])
                st_batch[bi] = {"xt": xt, "base": lo}
                st[g] = st_batch[bi]

            st_batch = {}

            def stage0(g):
                fd = _groups[g]
                s = dict(st[g])
                st[g] = s
                xo = _starts[g] - s["base"]
                w0_sb = batch_tiles["w0"]
                p0 = ps0.tile([D_HID, FD], F32, tag="p0")
                nc.tensor.matmul(p0[:, :fd], w0_sb,
                                 s["xt"][:, xo:xo + fd],
                                 start=True, stop=True)
                s["r1"], s["t1"] = exp_elu(g, 0, p0, fd, None)

            def stage1(g):
                fd = _groups[g]
                s = st[g]
                p1 = ps1.tile([D_HID, FD], F32, tag="p1")
                nc.tensor.matmul(p1[:, :fd], w1_sb, s["r1"][:, :fd],
                                 start=True, stop=False)
                nc.tensor.matmul(p1[:, :fd], w1_sb, s["t1"][:, :fd],
                                 start=False, stop=True)
                s["r2"], s["t2"] = exp_elu(g, 1, p1, fd, nb1_sb)

            def stage2(g):
                fd = _groups[g]
                s = st.pop(g)
                p2 = ps2.tile([D_OUT, FD], F32, tag="p2")
                nc.tensor.matmul(p2[:, :fd], w2_sb, s["r2"][:, :fd],
                                 start=True, stop=False)
                nc.tensor.matmul(p2[:, :fd], w2_sb, s["t2"][:, :fd],
                                 start=False, stop=True)
                o = sb.tile([D_OUT, FD], F16, tag="o")
                drain(o[:, :fd], p2[:, :fd], ncb2_sb,
                      g in OUT_DRAIN_ON_ACT)
                eng = nc.gpsimd if g % 2 == 0 else nc.sync
                eng.dma_start(yT[:, _starts[g]:_starts[g] + fd], o[:, :fd])

            # software-pipelined emission
            for gg in range(G + 3):
                if gg < G:
                    stage_load(gg)
                    if gg == 0:
                        # w0 lives in batch-0's tile, cols 0..95 of xw
                        batch_tiles["w0"] = st[0]["xt"][:, 0:D_HID]
                if 0 <= gg - 1 < G:
                    stage0(gg - 1)
                if 0 <= gg - 2 < G:
                    stage1(gg - 2)
                if 0 <= gg - 3 < G:
                    stage2(gg - 3)

    nc.compile()
    return nc


_prog_cache = []
last_result = None


def kernel(**inputs) -> np.ndarray:
    global last_result
    x = np.asarray(inputs["x"], np.float32)           # [50000, 128]
    W0 = np.asarray(inputs["W0"], np.float32).reshape(D_HID, D_IN)
    W1 = np.asarray(inputs["W1"], np.float32).reshape(D_HID, D_HID)
    W2 = np.asarray(inputs["W2"], np.float32).reshape(D_OUT, D_HID)

    n = x.shape[0]
    assert n == N_CORES * N_PER, f"unexpected node count {n}"

    import ml_dtypes
    xT16 = x.T.astype(np.float16)                            # [128, 50000]
    w0t = W0.T.astype(np.float16)                            # [128, 96]
    w1tb = W1.T.astype(ml_dtypes.bfloat16)                   # [96, 96]
    w2tb = W2.T.astype(ml_dtypes.bfloat16)                   # [96, 40]
    wb = np.ascontiguousarray(
        np.concatenate([w1tb, w2tb], axis=1))                # [96, 136]
    biasm = np.zeros((D_HID, 2), np.float32)
    biasm[:, 0] = -w1tb.astype(np.float32).sum(axis=0)       # -(W1 @ 1)
    biasm[:D_OUT, 1] = -w2tb.astype(np.float32).sum(axis=0)  # -(W2 @ 1)

    if not _prog_cache:
        _prog_cache.append(_build_program())
    nc = _prog_cache[0]

    in_maps = []
    for i in range(N_CORES):
        xw = np.ascontiguousarray(
            np.concatenate([w0t, xT16[:, i * N_PER:(i + 1) * N_PER]], axis=1))
        in_maps.append(dict(xw=xw, wb=wb, bias=biasm))
    res = run_bass_kernel_spmd(nc, in_maps, list(range(N_CORES)))
    last_result = res
    out = np.concatenate(
        [np.asarray(res.results[i]["yT"], np.float32).T for i in range(N_CORES)],
        axis=0,
    )
    return out


if __name__ == "__main__":
    data = np.load("/tmp/gat_inputs.npz")
    y = kernel(**{k: data[k] for k in data.files})
    print("out", y.shape, y.dtype, "absmax", np.abs(y).max())


# revision 20
# speedup vs baseline: 4.6726x; 1.2107x over previous
"""Trainium2 Bass kernel for nn_GAT_87617332838818.

Mathematical collapse: the reference GAT aggregates ``alpha * hp[:, dst]``
over incoming edges per destination node.  Since the softmax weights alpha
sum to exactly 1 within each destination segment and the aggregated message
``hp[dst]`` is constant within the segment, the whole message-passing step
is the identity: ``out[n] = hp[n]``.  The network therefore reduces to a
per-node 3-layer MLP:

    logits = W2r @ elu(W1r @ elu(W0r @ x^T))        (per node column)

with W0r = W0.reshape(96,128), W1r = W1.reshape(96,96), W2r = W2.reshape(40,96)
(head-concat order matches the plain reshape).  Verified numerically against
the reference: rel fro err 4e-7 in f32; 4.5e-3 with this device pipeline.

Device strategy (8 NeuronCores, node-sharded 6250 rows each):
  - activations kept feature-on-partition: xT [128, n], h [96, n]
  - ELU via the split  elu(p') + 1 = max(p',0) + min(exp(p'),1)  with
    p' = p + nb (nb folds the "+1" inflation of the previous layer:
    nb = -W @ ones).  r = max(p+nb,0) and t = min(exp(p+nb),1) are fed
    through TWO accumulating matmuls (linearity), so the inflated h+1 is
    only ever formed in f32 PSUM — bf16-safe.
  - final layer bias cb2 = W2 @ ones subtracted in the output drain pass.
  - per 512-column group: PE 5 matmuls, ACT exp x2 + relu drain, DVE
    r/t/out passes; PSUM drains split between DVE and ACT for balance.
  - 3-stage software-pipelined emission so each engine's in-order stream
    always has ready work (avoids head-of-line blocking across groups).
  - ~18 dummy matmuls parked in the DMA-bound head flip the PE HAM clock
    gate to 2.4 GHz before the real matmuls start (measured 427->216 ns).
  - w0 rides in the first x DMA batch; w1/w2 and biases are packed into
    single DMAs to cut ~620 ns/issue sequencer serialization.
"""

import os
import sys

import numpy as np

for _p in ("/root/.axon_site/_ro/trn_rl_repo", "/opt/trn_rl_repo"):
    if os.path.isdir(_p) and _p not in sys.path:
        sys.path.append(_p)

import concourse.bass as bass
import concourse.tile as tile
from concourse import bacc, mybir
from concourse.bass_utils import run_bass_kernel_spmd

N_CORES = 8
N_PER = 6250            # 50000 / 8
D_IN = 128
D_HID = 96
D_OUT = 40
FD = 512                # group free-dim (1 PSUM bank)

F16 = mybir.dt.float16
BF16 = mybir.dt.bfloat16
F32 = mybir.dt.float32

Act = mybir.ActivationFunctionType
Alu = mybir.AluOpType

_groups = [FD] * (N_PER // FD)
if N_PER % FD:
    _groups.append(N_PER % FD)
G = len(_groups)
_starts = [sum(_groups[:i]) for i in range(G)]

# drain assignment: which r/out PSUM drains go to ACT instead of DVE.
R_DRAIN_ON_ACT = tuple((g, 0) for g in range(G) if g % 4 != 3)
OUT_DRAIN_ON_ACT = ()
X_BATCHES = [1, 4, 4, 4]     # groups per input DMA (first small -> fast start)
N_WARMUP_MM = 18             # dummy matmuls to flip the PE HAM to 2.4 GHz

_batch_of = {}
_b0 = 0
for _bi, _bn in enumerate(X_BATCHES):
    for _g in range(_b0, min(_b0 + _bn, G)):
        _batch_of[_g] = _bi
    _b0 += _bn
assert _b0 >= G


def _build_program() -> bass.Bass:
    nc = bacc.Bacc(None, target_bir_lowering=False, debug=False)

    # xw packs [w0t | xT]: cols 0..95 = W0^T fp16, cols 96.. = x^T shard
    xw = nc.declare_dram_parameter("xw", [D_IN, D_HID + N_PER], F16,
                                   isOutput=False)
    # wb packs [w1t | w2t] bf16
    wb = nc.declare_dram_parameter("wb", [D_HID, D_HID + D_OUT], BF16,
                                   isOutput=False)
    # biases: col 0 = nb1 (= -W1@1), col 1 rows 0..39 = -cb2 (= -W2@1)
    bias = nc.declare_dram_parameter("bias", [104, 2], F32, isOutput=False)
    yT = nc.declare_dram_parameter("yT", [D_OUT, N_PER], F16, isOutput=True)

    st = {}  # per-group pipeline state
    batch_tiles = {}

    with tile.TileContext(nc) as tc:
        with (
            tc.tile_pool(name="consts", bufs=1) as consts,
            tc.tile_pool(name="x0", bufs=1) as x0pool,
            tc.tile_pool(name="xin", bufs=2) as xpool,
            tc.tile_pool(name="sb", bufs=3) as sb,
            tc.tile_pool(name="ps0", bufs=3, space="PSUM") as ps0,
            tc.tile_pool(name="ps1", bufs=3, space="PSUM") as ps1,
            tc.tile_pool(name="ps2", bufs=2, space="PSUM") as ps2,
        ):
            # --- PE warm-up: dummy matmuls on garbage SBUF during the
            # DMA-bound head; no data deps, output overwritten later.
            junk_w = consts.tile([D_IN, D_OUT], F16, tag="junkw")
            junk_x = consts.tile([D_IN, FD], F16, tag="junkx")
            nc.gpsimd.memset(junk_w[:], 0.0)
            nc.gpsimd.memset(junk_x[:], 0.0)
            warm = ps2.tile([D_OUT, FD], F32, tag="p2")
            for _ in range(N_WARMUP_MM):
                nc.tensor.matmul(warm[:], junk_w[:], junk_x[:],
                                 start=True, stop=True)

            wb_sb = consts.tile([D_HID, D_HID + D_OUT], BF16, tag="wb")
            bias_sb = consts.tile([104, 2], F32, tag="bias")
            nc.sync.dma_start(wb_sb[:], wb[:])
            nc.sync.dma_start(bias_sb[:], bias[:])
            w1_sb = wb_sb[:, :D_HID]
            w2_sb = wb_sb[:, D_HID:D_HID + D_OUT]
            nb1_sb = bias_sb[:D_HID, 0:1]
            ncb2d_sb = bias_sb[:104, 1:2]   # -cb2 at rows 0:40 and 64:104

            def drain(out_ap, psum_ap, bias_ap, on_act):
                """out = psum + bias, PSUM -> SBUF."""
                if on_act:
                    nc.scalar.activation(out_ap, psum_ap, Act.Identity,
                                         bias=bias_ap)
                else:
                    nc.vector.tensor_scalar_add(out_ap, psum_ap, bias_ap)

            def relu_drain(out_ap, psum_ap, bias_ap, on_act):
                """out = max(psum + bias, 0), PSUM -> SBUF bf16."""
                if on_act:
                    nc.scalar.activation(out_ap, psum_ap, Act.Relu,
                                         bias=(bias_ap if bias_ap is not None
                                               else 0.0))
                elif bias_ap is None:
                    nc.vector.tensor_scalar_max(out_ap, psum_ap, 0.0)
                else:
                    nc.vector.tensor_scalar(out_ap, psum_ap, bias_ap, 0.0,
                                            Alu.add, Alu.max)

            def exp_elu(g, lyr, p, fd, bias_ap):
                """From psum p: e=exp(p+nb), r=max(p+nb,0), t=min(e,1)."""
                e = sb.tile([D_HID, FD], BF16, tag=f"e{lyr}")
                if bias_ap is None:
                    nc.scalar.activation(e[:, :fd], p[:, :fd], Act.Exp)
                else:
                    nc.scalar.activation(e[:, :fd], p[:, :fd], Act.Exp,
                                         bias=bias_ap)
                r = sb.tile([D_HID, FD], BF16, tag=f"r{lyr}")
                relu_drain(r[:, :fd], p[:, :fd], bias_ap,
                           (g, lyr) in R_DRAIN_ON_ACT)
                t = sb.tile([D_HID, FD], BF16, tag=f"t{lyr}")
                nc.vector.tensor_scalar_min(t[:, :fd], e[:, :fd], 1.0)
                return r, t

            def stage_load(g):
                bi = _batch_of[g]
                if g > 0 and _batch_of[g - 1] == bi:
                    st[g] = st_batch[bi]
                    return
                g0 = g
                g1 = g0
                while g1 + 1 < G and _batch_of[g1 + 1] == bi:
                    g1 += 1
                lo = _starts[g0] + (0 if bi else -D_HID)  # batch 0 incl. w0
                hi = _starts[g1] + _groups[g1]
                cols = hi - lo
                pool = x0pool if bi == 0 else xpool
                width = D_HID + FD * X_BATCHES[0] if bi == 0 else FD * 4
                xt = pool.tile([D_IN, width], F16,
                               tag=("xt0" if bi == 0 else "xt"))
                nc.sync.dma_start(xt[:, :cols],
                                  xw[:, D_HID + lo:D_HID + hi])
                st_batch[bi] = {"xt": xt, "base": lo}
                st[g] = st_batch[bi]

            st_batch = {}

            def stage0(g):
                fd = _groups[g]
                s = dict(st[g])
                st[g] = s
                xo = _starts[g] - s["base"]
                w0_sb = batch_tiles["w0"]
                p0 = ps0.tile([D_HID, FD], F32, tag="p0")
                nc.tensor.matmul(p0[:, :fd], w0_sb,
                                 s["xt"][:, xo:xo + fd],
                                 start=True, stop=True)
                s["r1"], s["t1"] = exp_elu(g, 0, p0, fd, None)

            def stage1(g):
                fd = _groups[g]
                s = st[g]
                p1 = ps1.tile([D_HID, FD], F32, tag="p1")
                nc.tensor.matmul(p1[:, :fd], w1_sb, s["r1"][:, :fd],
                                 start=True, stop=False)
                nc.tensor.matmul(p1[:, :fd], w1_sb, s["t1"][:, :fd],
                                 start=False, stop=True)
                s["r2"], s["t2"] = exp_elu(g, 1, p1, fd, nb1_sb)

            pair_state = {}

            def stage2(g):
                # pairs of groups share one [80, FD] psum tile: group 2k in
                # partitions 0:40, group 2k+1 in 40:80 -> one drain per pair.
                fd = _groups[g]
                s = st.pop(g)
                if g % 2 == 0:
                    p2 = ps2.tile([104, FD], F32, tag="p2")
                    pair_state[g // 2] = p2
                    rows = slice(0, D_OUT)
                else:
                    p2 = pair_state[g // 2]
                    rows = slice(64, 64 + D_OUT)
                nc.tensor.matmul(p2[rows, :fd], w2_sb, s["r2"][:, :fd],
                                 start=True, stop=False)
                nc.tensor.matmul(p2[rows, :fd], w2_sb, s["t2"][:, :fd],
                                 start=False, stop=True)
                last_of_pair = (g % 2 == 1) or (g == G - 1)
                if not last_of_pair:
                    return
                nrows = 104 if g % 2 == 1 else D_OUT
                o = sb.tile([104, FD], F16, tag="o")
                drain(o[:nrows, :fd], p2[:nrows, :fd], ncb2d_sb[:nrows],
                      g in OUT_DRAIN_ON_ACT)
                ga = g - 1 if g % 2 == 1 else g
                fda = _groups[ga]
                nc.gpsimd.dma_start(yT[:, _starts[ga]:_starts[ga] + fda],
                                    o[0:D_OUT, :fda])
                if g % 2 == 1:
                    nc.sync.dma_start(yT[:, _starts[g]:_starts[g] + fd],
                                      o[64:64 + D_OUT, :fd])

            # software-pipelined emission
            for gg in range(G + 3):
                if gg < G:
                    stage_load(gg)
                    if gg == 0:
                        # w0 lives in batch-0's tile, cols 0..95 of xw
                        batch_tiles["w0"] = st[0]["xt"][:, 0:D_HID]
                if 0 <= gg - 1 < G:
                    stage0(gg - 1)
                if 0 <= gg - 2 < G:
                    stage1(gg - 2)
                if 0 <= gg - 3 < G:
                    stage2(gg - 3)

    nc.compile()
    return nc


_prog_cache = []
last_result = None


def kernel(**inputs) -> np.ndarray:
    global last_result
    x = np.asarray(inputs["x"], np.float32)           # [50000, 128]
    W0 = np.asarray(inputs["W0"], np.float32).reshape(D_HID, D_IN)
    W1 = np.asarray(inputs["W1"], np.float32).reshape(D_HID, D_HID)
    W2 = np.asarray(inputs["W2"], np.float32).reshape(D_OUT, D_HID)

    n = x.shape[0]
    assert n == N_CORES * N_PER, f"unexpected node count {n}"

    import ml_dtypes
    xT16 = x.T.astype(np.float16)                            # [128, 50000]
    w0t = W0.T.astype(np.float16)                            # [128, 96]
    w1tb = W1.T.astype(ml_dtypes.bfloat16)                   # [96, 96]
    w2tb = W2.T.astype(ml_dtypes.bfloat16)                   # [96, 40]
    wb = np.ascontiguousarray(
        np.concatenate([w1tb, w2tb], axis=1))                # [96, 136]
    biasm = np.zeros((104, 2), np.float32)
    biasm[:D_HID, 0] = -w1tb.astype(np.float32).sum(axis=0)  # -(W1 @ 1)
    ncb2 = -w2tb.astype(np.float32).sum(axis=0)              # -(W2 @ 1)
    biasm[:D_OUT, 1] = ncb2
    biasm[64:64 + D_OUT, 1] = ncb2                           # replicated

    if not _prog_cache:
        _prog_cache.append(_build_program())
    nc = _prog_cache[0]

    in_maps = []
    for i in range(N_CORES):
        xw = np.ascontiguousarray(
            np.concatenate([w0t, xT16[:, i * N_PER:(i + 1) * N_PER]], axis=1))
        in_maps.append(dict(xw=xw, wb=wb, bias=biasm))
    res = run_bass_kernel_spmd(nc, in_maps, list(range(N_CORES)))
    last_result = res
    out = np.concatenate(
        [np.asarray(res.results[i]["yT"], np.float32).T for i in range(N_CORES)],
        axis=0,
    )
    return out


if __name__ == "__main__":
    data = np.load("/tmp/gat_inputs.npz")
    y = kernel(**{k: data[k] for k in data.files})
    print("out", y.shape, y.dtype, "absmax", np.abs(y).max())
